# revision 39
# baseline (speedup 1.0000x reference)
"""Multi-head causal self-attention on 8 Trainium2 NeuronCores.

Problem: x [4, 2048, 1024], Wq/Wk/Wv/Wo [1024, 1024] (applied as x @ W.T),
16 heads, dk=64, causal softmax, output [4, 2048, 1024], all fp32.

Sharding: 8 cores = 4 batches x 2 head-groups (8 heads each).
Each core computes QKV projections for its 8 heads, streaming causal
attention, and a partial output projection (Wo row-split). The host adds
the two partial outputs per batch element.

Per-core layouts (chosen so NO on-device transposes are needed):
  xT  [1024, 2048]  = x[b].T          (host-transposed)
  wqT [1024, 512]   = (Wq/8).T cols for this head group (1/sqrt(dk) folded)
  wkT [1024, 512], wvT [1024, 512]
  woT [512, 1024]   = Wo[:, cols].T
  QT/KT on chip as [feat, seq] (head pairs stacked on partitions),
  V as [seq, feat] bf16. scoresT tiles [k=128, q=512] per head pair are
  exp'ed on ScalarE into bf16; the causal mask is applied with
  affine_select on the idle GpSimd engine; the softmax denominator is a
  ones-matmul (partition reduction on the PE); 1/l is broadcast across
  partitions with a tiny constant matmul.

Projection s-chunks and attention q-blocks are interleaved in program
order so TensorE (projections) and ScalarE (exp) work concurrently.
"""

import ml_dtypes
import numpy as np

import concourse.bass as bass
import concourse.mybir as mybir
import concourse.tile as tile
from concourse.bass_utils import run_bass_kernel_spmd
from concourse.vector_clock import ScopedClock

F32 = mybir.dt.float32
F32R = mybir.dt.float32r
BF16 = mybir.dt.bfloat16
AF = mybir.ActivationFunctionType
ALU = mybir.AluOpType

B, S, D = 4, 2048, 1024
H = 16
DK = 64
N_CORES = 8
HG = 512          # head-group width (8 heads x 64)


# ---------------------------------------------------------------------------
# This walrus accepts at most 1 sem wait per instruction (2 for
# EventSemaphore). Tile emits more in two places; both are fixed up here by
# moving excess waits onto preceding instructions on the same engine.
# ---------------------------------------------------------------------------
def _split_drain_and_barrier(self, tick_clock, wait_clock):
    nc = self.nc
    probe = nc.sync.nop(nofuse=True, hint="tile_drain_waits")
    wait_clock.add_sem_waits(
        probe.ins, ScopedClock({None: tick_clock.global_clock})
    )
    si = probe.ins.sync_info
    waits = list(si.on_wait) if si is not None else []
    if len(waits) > 1:
        probe.ins.sync_info = mybir.SyncInfo(on_wait=[waits[0]], on_update=[])
        for w in waits[1:]:
            n = nc.sync.nop(nofuse=True, hint="tile_drain_waits")
            n.ins.sync_info = mybir.SyncInfo(on_wait=[w], on_update=[])
    nc.sync.drain()
    nc.all_engine_barrier()
    popped = nc._tile_sem_poison_stack.pop()
    assert popped is self._sem_poison
    nc.clear_and_free_semaphores(list(self.sems.allocated().values()))
    nc.all_engine_barrier()


tile.TileContext._drain_and_barrier = _split_drain_and_barrier

_wsplit_counter = [0]


def _enforce_wait_limits(m):
    for fn in m.functions:
        for bb in fn.blocks:
            out = []
            changed = False
            for inst in bb.instructions:
                si = inst.sync_info
                cap = 2 if isinstance(inst, mybir.InstEventSemaphore) else 1
                if si is not None and len(si.on_wait) > cap:
                    waits = list(si.on_wait)
                    keep, extra = waits[:cap], waits[cap:]
                    for i in range(0, len(extra), 2):
                        _wsplit_counter[0] += 1
                        out.append(mybir.InstEventSemaphore(
                            name=f"I-wsplit-{_wsplit_counter[0]}",
                            engine=inst.engine,
                            ins=[], outs=[],
                            sync_info=mybir.SyncInfo(
                                on_wait=extra[i:i + 2], on_update=[]),
                        ))
                    inst.sync_info = mybir.SyncInfo(
                        on_wait=keep, on_update=list(si.on_update))
                    changed = True
                out.append(inst)
            if changed:
                bb.instructions = out


def build_nc():
    nc = bass.Bass()

    xT = nc.declare_dram_parameter("xT", [D, S], BF16, isOutput=False)
    wqT = nc.declare_dram_parameter("wqT", [D, HG], BF16, isOutput=False)
    wkT = nc.declare_dram_parameter("wkT", [D, HG], BF16, isOutput=False)
    wvT = nc.declare_dram_parameter("wvT", [D, HG], BF16, isOutput=False)
    woT = nc.declare_dram_parameter("woT", [HG, D], BF16, isOutput=False)
    yout = nc.declare_dram_parameter("y", [S, D], F32, isOutput=True)

    KT8 = D // 128   # contraction tiles for the projections
    NP = 4           # head pairs per core
    NS = S // 128    # seq tiles of 128

    from contextlib import ExitStack

    with tile.TileContext(nc) as tc, ExitStack() as ctx:
        ep = ctx.enter_context
        consts = ep(tc.tile_pool(name="consts", bufs=1))
        qt_pool = ep(tc.tile_pool(name="qt", bufs=1))
        kt_pool = ep(tc.tile_pool(name="kt", bufs=1))
        v_pool = ep(tc.tile_pool(name="v", bufs=1))
        wo_pool = ep(tc.tile_pool(name="wo", bufs=1))
        wq_pool = ep(tc.tile_pool(name="wq", bufs=1))
        wk_pool = ep(tc.tile_pool(name="wk", bufs=1))
        wv_pool = ep(tc.tile_pool(name="wv", bufs=1))
        xt_pool = ep(tc.tile_pool(name="xt", bufs=2))
        exp_pool = ep(tc.tile_pool(name="exp", bufs=5))
        ctxn_pool = ep(tc.tile_pool(name="ctxn", bufs=12))
        rcp_pool = ep(tc.tile_pool(name="rcp", bufs=2))
        ctxraw_pool = ep(tc.tile_pool(name="ctxraw", bufs=2))
        ybuf_pool = ep(tc.tile_pool(name="ybuf", bufs=2))
        mm_ps = ep(tc.tile_pool(name="mm_ps", bufs=2, space="PSUM"))
        sc_ps = ep(tc.tile_pool(name="sc_ps", bufs=2, space="PSUM"))
        ctx_ps = ep(tc.tile_pool(name="ctx_ps", bufs=1, space="PSUM"))

        # ---- constants and weights ----------------------------------------
        # 1/l broadcast selector: picks rcp row 64 (head a's l) into output
        # partitions 0:64 and rcp row 0 (head b's l) into partitions 64:128.
        # Memset can't write f32r directly, so memset bf16 and cast on DVE.
        bcsel = consts.tile([65, 128], BF16, tag="bcsel")
        nc.gpsimd.memset(bcsel[:], 0.0)
        nc.gpsimd.memset(bcsel[64:65, 0:64], 1.0)
        nc.gpsimd.memset(bcsel[0:1, 64:128], 1.0)
        # One-time seed of the ln-staging buffers: rows 1..63 are never
        # written (Ln only writes rows 0 and 64), and exp(-x) of them must
        # stay finite; bcsel zeroes them in the contraction.
        for _ in range(2):
            w = rcp_pool.tile([65, 512], F32, tag="ln", name="lnwarm0")
            nc.vector.memset(w[0:64, :], 0.0)

        QT = [qt_pool.tile([128, S], BF16, tag=f"qt{p}", name=f"QT{p}")
              for p in range(NP)]
        KTt = [kt_pool.tile([128, S], BF16, tag=f"kt{p}", name=f"KTt{p}")
               for p in range(NP)]
        # V2: per seq-tile, 4 pair-blocks of 256 cols. Pair block layout:
        #   cols   0:64  Va   | col  64 ones | cols  65:128 zeros
        #   col  128 ones | cols 129:192 zeros | cols 192:256 Vb
        # so the ctx matmul lhsT [128,128] slices fuse the softmax denominator
        # into the context accumulation: head a -> ctx at out partitions 0:64,
        # l_a at 64; head b -> l_b at 0, ctx at 64:128.
        V2 = [v_pool.tile([128, 1024], BF16, tag=f"v{s}", name=f"V{s}")
              for s in range(NS)]
        # DMA order matters for startup latency: the first projection
        # psum-group needs wq + chunk-0 x tiles, so those go first; wo is not
        # needed until the first output projection (~80us in) and goes last.
        wo_t = []
        wq_t, wk_t, wv_t = [], [], []
        for kt in range(KT8):
            t = wq_pool.tile([128, HG], BF16, tag=f"w{kt}", name=f"wq{kt}")
            nc.sync.dma_start(t[:], wqT[kt * 128:(kt + 1) * 128, :])
            wq_t.append(t)

        def emit_xt_dmas(st):
            xts = []
            eng = nc.gpsimd if st == 0 else nc.sync
            for kt in range(KT8):
                t = xt_pool.tile([128, 512], BF16, tag=f"xt{kt}",
                                 name=f"xt{st}_{kt}")
                eng.dma_start(
                    t[:], xT[kt * 128:(kt + 1) * 128, st * 512:(st + 1) * 512]
                )
                xts.append(t)
            return xts

        def proj_items(st, xts):
            """QKV projection work for chunk st as a flat list of closures,
            one instruction each, so they can be sprinkled between attention
            triples at fine grain."""
            items = []

            def qk_group(ot, w_t, dst, name):
                holder = {}

                def mk_mm(kt):
                    def go():
                        if "ps" not in holder:
                            holder["ps"] = mm_ps.tile(
                                [128, 512], F32, tag="mm", name=name)
                        nc.tensor.matmul(
                            holder["ps"][:],
                            w_t[kt][:, ot * 128:(ot + 1) * 128],
                            xts[kt][:],
                            start=(kt == 0),
                            stop=(kt == KT8 - 1),
                        )
                    return go

                def copy():
                    nc.vector.tensor_copy(
                        dst[ot][:, st * 512:(st + 1) * 512], holder["ps"][:])

                return [mk_mm(kt) for kt in range(KT8)] + [copy]

            def v_group(sub):
                holder = {}

                def mk_mm(kt):
                    def go():
                        if "ps" not in holder:
                            holder["ps"] = mm_ps.tile(
                                [128, 512], F32, tag="mm", name=f"pv{st}{sub}")
                        nc.tensor.matmul(
                            holder["ps"][:],
                            xts[kt][:, sub * 128:(sub + 1) * 128],
                            wv_t[kt][:],
                            start=(kt == 0),
                            stop=(kt == KT8 - 1),
                        )
                    return go

                def masks():
                    # ones/zeros padding of the V2 pair blocks (once per tile)
                    v2 = V2[st * 4 + sub]
                    vv = v2[:].rearrange("p (pr h m) -> p pr h m", pr=4, h=2)
                    nc.gpsimd.memset(vv[:, :, 0, 64:128], 0.0)
                    nc.gpsimd.memset(vv[:, :, 1, 0:64], 0.0)
                    nc.gpsimd.memset(vv[:, :, 0, 64:65], 1.0)
                    nc.gpsimd.memset(vv[:, :, 1, 0:1], 1.0)

                def copy():
                    v2 = V2[st * 4 + sub]
                    vv = v2[:].rearrange("p (pr h m) -> p pr h m", pr=4, h=2)
                    src = holder["ps"][:].rearrange(
                        "p (pr h c) -> p pr h c", pr=4, h=2)
                    nc.vector.tensor_copy(vv[:, :, 0, 0:64], src[:, :, 0, :])
                    nc.vector.tensor_copy(vv[:, :, 1, 64:128], src[:, :, 1, :])

                return [masks] + [mk_mm(kt) for kt in range(KT8)] + [copy]

            for ot in range(NP):
                items.extend(qk_group(ot, wq_t, QT, f"pq{st}{ot}"))
                items.extend(qk_group(ot, wk_t, KTt, f"pk{st}{ot}"))
            for sub in range(4):
                items.extend(v_group(sub))
            return items

        def drain_pair(rec):
            """Drain a finished pair's ctx PSUM banks: ctx halves to SBUF
            (lane-aligned by construction), l rows via ScalarE exp(-ln(l)).
            Invoked after the NEXT pair's first scores are emitted so these
            ops overlap its first exp window."""
            ctxA, ctxB, label = rec["ctxA"], rec["ctxB"], rec["label"]
            cnsrc = ctxraw_pool.tile([128, 512], F32, tag="cr",
                                     name=f"cr{label}")
            lnb = rcp_pool.tile([65, 512], F32, tag="ln", name=f"ln{label}")
            nc.scalar.activation(lnb[64:65, :], ctxA[64:65, :], AF.Ln)
            nc.scalar.activation(lnb[0:1, :], ctxB[0:1, :], AF.Ln)
            nc.vector.tensor_copy(cnsrc[0:64, :], ctxA[0:64, :])
            nc.vector.tensor_copy(cnsrc[64:128, :], ctxB[64:128, :])
            rcp = rcp_pool.tile([65, 512], BF16, tag="rcp",
                                name=f"rcp{label}")
            nc.scalar.activation(rcp[:], lnb[:], AF.Exp, scale=-1.0)
            return (rcp, cnsrc, rec["label"], rec["sink"])

        def norm_part2(rcp, cnsrc, label, sink):
            """Broadcast 1/l across partitions (selector matmul) and scale
            the packed ctx tile straight off the broadcast PSUM."""
            bcp = mm_ps.tile([128, 512], F32, tag="mm", name=f"bcp{label}")
            nc.tensor.matmul(bcp[:], bcsel[:], rcp[:], start=True, stop=True)
            cn = ctxn_pool.tile([128, 512], BF16, tag="cn", name=f"cn{label}")
            nc.vector.tensor_mul(cn[:], cnsrc[:], bcp[:])
            sink.append(cn)

        def attention_block(j, fill, drainq, carried_norm, sink):
            """Causal attention + partial output projection for q-tile j.
            `fill` is a list of closures (next chunk's projection groups)
            sprinkled into the PE stream to cover exp-wait stalls.
            `carried_norm` is the previous block's unemitted normalize; the
            one left over here is returned for the next block, so the PE
            stream never stalls on a normalize chain at a block boundary."""
            fill = list(fill)
            n_triples = NP * 4 * (j + 1)
            per_triple = -(-len(fill) // n_triples) if fill else 0

            def emit_fill(n):
                for _ in range(n):
                    if not fill:
                        return
                    if fill[0]() is False:
                        return  # head item's inputs not produced yet
                    fill.pop(0)

            def scores(pair, j, i):
                sc = sc_ps.tile([128, 1024], F32, tag="sc",
                                name=f"sc{j}{pair}{i}")
                qa = QT[pair][0:64, j * 512:(j + 1) * 512]
                qb = QT[pair][64:128, j * 512:(j + 1) * 512]
                ka = KTt[pair][0:64, i * 128:(i + 1) * 128]
                kb = KTt[pair][64:128, i * 128:(i + 1) * 128]
                nc.tensor.matmul(
                    sc[:, 0:512], ka, qa,
                    start=True, stop=True, tile_position=(0, 0),
                )
                nc.tensor.matmul(
                    sc[:, 512:1024], kb, qb,
                    start=True, stop=True, tile_position=(64, 0),
                )
                return sc

            ctxn = sink
            ni = 4 * (j + 1)

            def emit_ctx(ctxA, ctxB, et, i):
                first, last = (i == 0), (i == ni - 1)
                va = V2[i][:, pair * 256:pair * 256 + 128]
                vb = V2[i][:, pair * 256 + 128:pair * 256 + 256]
                nc.tensor.matmul(ctxA[:], va, et[:, 0:512],
                                 start=first, stop=last)
                nc.tensor.matmul(ctxB[:], vb, et[:, 512:1024],
                                 start=first, stop=last)

            def emit_exp(sc, i):
                et = exp_pool.tile([128, 1024], BF16, tag="exp",
                                   name=f"et{j}{pair}{i}")
                nc.scalar.activation(et[:], sc[:], AF.Exp)
                if i >= 4 * j:
                    # diagonal block: zero the future positions
                    # keep et[kk, h, qq] iff qq - kk - 128*(i-4j) >= 0
                    p = i - 4 * j
                    nc.gpsimd.affine_select(
                        out=et[:], in_=et[:],
                        pattern=[[0, 2], [1, 512]],
                        compare_op=ALU.is_ge,
                        fill=0.0,
                        base=-128 * p,
                        channel_multiplier=-1,
                    )
                return et

            # i-tiles are processed in groups of two so the scores quadrant
            # phase is entered half as often, and each group's PE stream is
            # [ctx pair x2 | fills | scores x2] — ctx and fills merge into
            # one uninterrupted full-array run.
            pending_norm = carried_norm
            for pair in range(NP):
                ctxA = ctx_ps.tile([128, 512], F32, tag="ctxA",
                                   name=f"ctxA{j}{pair}")
                ctxB = ctx_ps.tile([128, 512], F32, tag="ctxB",
                                   name=f"ctxB{j}{pair}")
                scs = [scores(pair, j, 0), scores(pair, j, 1)]
                # drain the previous pair's ctx banks here, AFTER this pair's
                # first scores are in the PE queue, then emit the normalize
                # for the pair before that
                if drainq:
                    rec = drainq.pop(0)
                    new_norm = drain_pair(rec)
                    if pending_norm is not None:
                        norm_part2(*pending_norm)
                    pending_norm = new_norm
                pending = []
                for ig in range(0, ni, 2):
                    pending.append((emit_exp(scs[0], ig), ig))
                    pending.append((emit_exp(scs[1], ig + 1), ig + 1))
                    while len(pending) > 2:
                        emit_ctx(ctxA, ctxB, *pending.pop(0))
                    emit_fill(2 * per_triple + (per_triple if ig == 0 else 0))
                    if ig + 2 < ni:
                        scs = [scores(pair, j, ig + 2),
                               scores(pair, j, ig + 3)]
                while pending:
                    emit_ctx(ctxA, ctxB, *pending.pop(0))
                    emit_fill(per_triple)
                drainq.append(dict(ctxA=ctxA, ctxB=ctxB,
                                   label=f"{j}{pair}", sink=ctxn))

            # drain any remaining fill that's ready; items still blocked on
            # the not-yet-flushed drain queue are returned to the caller
            while fill and fill[0]() is not False:
                fill.pop(0)
            return pending_norm, fill

        def outproj_items(j, ctxn):
            """Output projection for q-tile j as fine-grain fill items."""
            items = []

            def group(s4, oh, holder):
                def mk_mm(pair):
                    def go():
                        if len(ctxn) <= pair:
                            return False  # cn not normalized yet
                        if "ps" not in holder:
                            holder["ps"] = mm_ps.tile(
                                [128, 512], F32, tag="mm", name=f"yp{j}{s4}{oh}")
                        nc.tensor.matmul(
                            holder["ps"][:],
                            ctxn[pair][:, s4 * 128:(s4 + 1) * 128],
                            wo_t[pair][:, oh * 512:(oh + 1) * 512],
                            start=(pair == 0),
                            stop=(pair == NP - 1),
                        )
                    return go

                def copy():
                    nc.vector.tensor_copy(
                        holder["yb"][:, oh * 512:(oh + 1) * 512], holder["ps"][:])
                    del holder["ps"]

                return [mk_mm(p) for p in range(NP)] + [copy]

            for s4 in range(4):
                srow = j * 4 + s4
                holder = {}

                def alloc_yb(holder=holder, s4=s4):
                    holder["yb"] = ybuf_pool.tile(
                        [128, D], F32, tag="yb", name=f"yb{j}{s4}")

                items.append(alloc_yb)
                for oh in range(2):
                    items.extend(group(s4, oh, holder))

                def dma_out(holder=holder, srow=srow):
                    nc.sync.dma_start(
                        yout[srow * 128:(srow + 1) * 128, :], holder["yb"][:])

                items.append(dma_out)
            return items

        # chunk 0 projections run alone; attention block j then carries
        # chunk j+1's projections and block j-1's output projection as PE
        # filler for its exp-wait stalls.
        xts0 = emit_xt_dmas(0)
        for kt in range(KT8):
            for pool, lst, srcp, nm in (
                (wk_pool, wk_t, wkT, "wk"),
                (wv_pool, wv_t, wvT, "wv"),
            ):
                t = pool.tile([128, HG], BF16, tag=f"w{kt}", name=f"{nm}{kt}")
                nc.sync.dma_start(t[:], srcp[kt * 128:(kt + 1) * 128, :])
                lst.append(t)
        for item in proj_items(0, xts0):
            item()
        wo_t.clear()
        for c in range(NP):
            t = wo_pool.tile([128, D], BF16, tag=f"wo{c}")
            nc.sync.dma_start(t[:], woT[c * 128:(c + 1) * 128, :])
            wo_t.append(t)
        prev_out = []
        drainq = []
        carried = None
        sinks = [[] for _ in range(4)]
        for j in range(4):
            fill = list(prev_out)
            if j + 1 < 4:
                xts = emit_xt_dmas(j + 1)
                fill = proj_items(j + 1, xts) + fill
            else:
                # last block: interleave its own output projection as fill
                # (items block until each pair's cn lands, then emit)
                fill = fill + outproj_items(j, sinks[j])
            carried, leftover = attention_block(
                j, fill, drainq, carried, sinks[j])
            if j + 1 < 4:
                assert not leftover
                prev_out = outproj_items(j, sinks[j])
            else:
                prev_out = leftover
        # flush: the last pair's drain and the final two normalizes
        while drainq:
            new_norm = drain_pair(drainq.pop(0))
            if carried is not None:
                norm_part2(*carried)
            carried = new_norm
        if carried is not None:
            norm_part2(*carried)
        for item in prev_out:
            assert item() is not False


    _enforce_wait_limits(nc.m)
    return nc


_NC = None


def _get_nc():
    global _NC
    if _NC is None:
        _NC = build_nc()
    return _NC


def run(x, Wq, Wk, Wv, Wo, trace=False, trace_kwargs=None):
    """Returns (y, BassKernelResults)."""
    x = np.asarray(x, np.float32)
    scale = 1.0 / np.sqrt(DK)
    in_maps = []
    for core in range(N_CORES):
        b, g = core // 2, core % 2
        cols = slice(g * HG, (g + 1) * HG)
        bf = ml_dtypes.bfloat16
        in_maps.append({
            "xT": np.ascontiguousarray(x[b].T).astype(bf),
            "wqT": np.ascontiguousarray(
                np.asarray(Wq, np.float32).T[:, cols] * scale).astype(bf),
            "wkT": np.ascontiguousarray(
                np.asarray(Wk, np.float32).T[:, cols]).astype(bf),
            "wvT": np.ascontiguousarray(
                np.asarray(Wv, np.float32).T[:, cols]).astype(bf),
            "woT": np.ascontiguousarray(
                np.asarray(Wo, np.float32).T[cols, :]).astype(bf),
        })
    kw = dict(trace_kwargs or {})
    res = run_bass_kernel_spmd(
        _get_nc(), in_maps, list(range(N_CORES)), trace=trace, **kw
    )
    y = np.empty((B, S, D), np.float32)
    for b in range(B):
        y[b] = res.results[2 * b]["y"] + res.results[2 * b + 1]["y"]
    return y, res


def kernel(x, Wq, Wk, Wv, Wo):
    y, _ = run(x, Wq, Wk, Wv, Wo)
    return y



# revision 42
# speedup vs baseline: 1.0138x; 1.0138x over previous
"""Multi-head causal self-attention on 8 Trainium2 NeuronCores.

Problem: x [4, 2048, 1024], Wq/Wk/Wv/Wo [1024, 1024] (applied as x @ W.T),
16 heads, dk=64, causal softmax, output [4, 2048, 1024], all fp32.

Sharding: 8 cores = 4 batches x 2 head-groups (8 heads each).
Each core computes QKV projections for its 8 heads, streaming causal
attention, and a partial output projection (Wo row-split). The host adds
the two partial outputs per batch element.

Per-core layouts (chosen so NO on-device transposes are needed):
  xT  [1024, 2048]  = x[b].T          (host-transposed)
  wqT [1024, 512]   = (Wq/8).T cols for this head group (1/sqrt(dk) folded)
  wkT [1024, 512], wvT [1024, 512]
  woT [512, 1024]   = Wo[:, cols].T
  QT/KT on chip as [feat, seq] (head pairs stacked on partitions),
  V as [seq, feat] bf16. scoresT tiles [k=128, q=512] per head pair are
  exp'ed on ScalarE into bf16; the causal mask is applied with
  affine_select on the idle GpSimd engine; the softmax denominator is a
  ones-matmul (partition reduction on the PE); 1/l is broadcast across
  partitions with a tiny constant matmul.

Projection s-chunks and attention q-blocks are interleaved in program
order so TensorE (projections) and ScalarE (exp) work concurrently.
"""

import ml_dtypes
import numpy as np

import concourse.bass as bass
import concourse.mybir as mybir
import concourse.tile as tile
from concourse.bass_utils import run_bass_kernel_spmd
from concourse.vector_clock import ScopedClock

F32 = mybir.dt.float32
F32R = mybir.dt.float32r
BF16 = mybir.dt.bfloat16
AF = mybir.ActivationFunctionType
ALU = mybir.AluOpType

B, S, D = 4, 2048, 1024
H = 16
DK = 64
N_CORES = 8
HG = 512          # head-group width (8 heads x 64)


# ---------------------------------------------------------------------------
# This walrus accepts at most 1 sem wait per instruction (2 for
# EventSemaphore). Tile emits more in two places; both are fixed up here by
# moving excess waits onto preceding instructions on the same engine.
# ---------------------------------------------------------------------------
def _split_drain_and_barrier(self, tick_clock, wait_clock):
    nc = self.nc
    probe = nc.sync.nop(nofuse=True, hint="tile_drain_waits")
    wait_clock.add_sem_waits(
        probe.ins, ScopedClock({None: tick_clock.global_clock})
    )
    si = probe.ins.sync_info
    waits = list(si.on_wait) if si is not None else []
    if len(waits) > 1:
        probe.ins.sync_info = mybir.SyncInfo(on_wait=[waits[0]], on_update=[])
        for w in waits[1:]:
            n = nc.sync.nop(nofuse=True, hint="tile_drain_waits")
            n.ins.sync_info = mybir.SyncInfo(on_wait=[w], on_update=[])
    nc.sync.drain()
    nc.all_engine_barrier()
    popped = nc._tile_sem_poison_stack.pop()
    assert popped is self._sem_poison
    nc.clear_and_free_semaphores(list(self.sems.allocated().values()))
    nc.all_engine_barrier()


tile.TileContext._drain_and_barrier = _split_drain_and_barrier

_wsplit_counter = [0]


def _enforce_wait_limits(m):
    for fn in m.functions:
        for bb in fn.blocks:
            out = []
            changed = False
            for inst in bb.instructions:
                si = inst.sync_info
                cap = 2 if isinstance(inst, mybir.InstEventSemaphore) else 1
                if si is not None and len(si.on_wait) > cap:
                    waits = list(si.on_wait)
                    keep, extra = waits[:cap], waits[cap:]
                    for i in range(0, len(extra), 2):
                        _wsplit_counter[0] += 1
                        out.append(mybir.InstEventSemaphore(
                            name=f"I-wsplit-{_wsplit_counter[0]}",
                            engine=inst.engine,
                            ins=[], outs=[],
                            sync_info=mybir.SyncInfo(
                                on_wait=extra[i:i + 2], on_update=[]),
                        ))
                    inst.sync_info = mybir.SyncInfo(
                        on_wait=keep, on_update=list(si.on_update))
                    changed = True
                out.append(inst)
            if changed:
                bb.instructions = out


def build_nc():
    nc = bass.Bass()

    xT = nc.declare_dram_parameter("xT", [D, S], BF16, isOutput=False)
    wqT = nc.declare_dram_parameter("wqT", [D, HG], BF16, isOutput=False)
    wkT = nc.declare_dram_parameter("wkT", [D, HG], BF16, isOutput=False)
    wvT = nc.declare_dram_parameter("wvT", [D, HG], BF16, isOutput=False)
    woT = nc.declare_dram_parameter("woT", [HG, D], BF16, isOutput=False)
    yout = nc.declare_dram_parameter("y", [S, D], F32, isOutput=True)

    KT8 = D // 128   # contraction tiles for the projections
    NP = 4           # head pairs per core
    NS = S // 128    # seq tiles of 128

    from contextlib import ExitStack

    with tile.TileContext(nc) as tc, ExitStack() as ctx:
        ep = ctx.enter_context
        consts = ep(tc.tile_pool(name="consts", bufs=1))
        qt_pool = ep(tc.tile_pool(name="qt", bufs=1))
        kt_pool = ep(tc.tile_pool(name="kt", bufs=1))
        v_pool = ep(tc.tile_pool(name="v", bufs=1))
        wo_pool = ep(tc.tile_pool(name="wo", bufs=1))
        wq_pool = ep(tc.tile_pool(name="wq", bufs=1))
        wk_pool = ep(tc.tile_pool(name="wk", bufs=1))
        wv_pool = ep(tc.tile_pool(name="wv", bufs=1))
        xt_pool = ep(tc.tile_pool(name="xt", bufs=2))
        exp_pool = ep(tc.tile_pool(name="exp", bufs=5))
        ctxn_pool = ep(tc.tile_pool(name="ctxn", bufs=12))
        rcp_pool = ep(tc.tile_pool(name="rcp", bufs=2))
        ctxraw_pool = ep(tc.tile_pool(name="ctxraw", bufs=2))
        ybuf_pool = ep(tc.tile_pool(name="ybuf", bufs=2))
        mm_ps = ep(tc.tile_pool(name="mm_ps", bufs=2, space="PSUM"))
        sc_ps = ep(tc.tile_pool(name="sc_ps", bufs=2, space="PSUM"))
        ctx_ps = ep(tc.tile_pool(name="ctx_ps", bufs=1, space="PSUM"))

        # ---- constants and weights ----------------------------------------
        # 1/l broadcast selector: picks rcp row 64 (head a's l) into output
        # partitions 0:64 and rcp row 0 (head b's l) into partitions 64:128.
        # Memset can't write f32r directly, so memset bf16 and cast on DVE.
        bcsel = consts.tile([65, 128], BF16, tag="bcsel")
        nc.gpsimd.memset(bcsel[:], 0.0)
        nc.gpsimd.memset(bcsel[64:65, 0:64], 1.0)
        nc.gpsimd.memset(bcsel[0:1, 64:128], 1.0)
        # One-time seed of the ln-staging buffers: rows 1..63 are never
        # written (Ln only writes rows 0 and 64), and exp(-x) of them must
        # stay finite; bcsel zeroes them in the contraction.
        for _ in range(2):
            w = rcp_pool.tile([65, 512], F32, tag="ln", name="lnwarm0")
            nc.vector.memset(w[0:64, :], 0.0)

        QT = [qt_pool.tile([128, S], BF16, tag=f"qt{p}", name=f"QT{p}")
              for p in range(NP)]
        KTt = [kt_pool.tile([128, S], BF16, tag=f"kt{p}", name=f"KTt{p}")
               for p in range(NP)]
        # V2: per seq-tile, 4 pair-blocks of 256 cols. Pair block layout:
        #   cols   0:64  Va   | col  64 ones | cols  65:128 zeros
        #   col  128 ones | cols 129:192 zeros | cols 192:256 Vb
        # so the ctx matmul lhsT [128,128] slices fuse the softmax denominator
        # into the context accumulation: head a -> ctx at out partitions 0:64,
        # l_a at 64; head b -> l_b at 0, ctx at 64:128.
        V2 = [v_pool.tile([128, 1024], BF16, tag=f"v{s}", name=f"V{s}")
              for s in range(NS)]
        # DMA order matters for startup latency: the first projection
        # psum-group needs wq + chunk-0 x tiles, so those go first; wo is not
        # needed until the first output projection (~80us in) and goes last.
        wo_t = []
        wq_t, wk_t, wv_t = [], [], []
        for kt in range(KT8):
            t = wq_pool.tile([128, HG], BF16, tag=f"w{kt}", name=f"wq{kt}")
            nc.sync.dma_start(t[:], wqT[kt * 128:(kt + 1) * 128, :])
            wq_t.append(t)

        def emit_xt_dmas(st):
            xts = []
            eng = nc.gpsimd if st == 0 else nc.sync
            for kt in range(KT8):
                t = xt_pool.tile([128, 512], BF16, tag=f"xt{kt}",
                                 name=f"xt{st}_{kt}")
                eng.dma_start(
                    t[:], xT[kt * 128:(kt + 1) * 128, st * 512:(st + 1) * 512]
                )
                xts.append(t)
            return xts

        def proj_items(st, xts):
            """QKV projection work for chunk st as a flat list of closures,
            one instruction each, so they can be sprinkled between attention
            triples at fine grain."""
            items = []

            def qk_group(ot, w_t, dst, name):
                holder = {}

                def mk_mm(kt):
                    def go():
                        if "ps" not in holder:
                            holder["ps"] = mm_ps.tile(
                                [128, 512], F32, tag="mm", name=name)
                        nc.tensor.matmul(
                            holder["ps"][:],
                            w_t[kt][:, ot * 128:(ot + 1) * 128],
                            xts[kt][:],
                            start=(kt == 0),
                            stop=(kt == KT8 - 1),
                        )
                    return go

                def copy():
                    nc.vector.tensor_copy(
                        dst[ot][:, st * 512:(st + 1) * 512], holder["ps"][:])

                return [mk_mm(kt) for kt in range(KT8)] + [copy]

            def v_group(sub):
                holder = {}

                def mk_mm(kt):
                    def go():
                        if "ps" not in holder:
                            holder["ps"] = mm_ps.tile(
                                [128, 512], F32, tag="mm", name=f"pv{st}{sub}")
                        nc.tensor.matmul(
                            holder["ps"][:],
                            xts[kt][:, sub * 128:(sub + 1) * 128],
                            wv_t[kt][:],
                            start=(kt == 0),
                            stop=(kt == KT8 - 1),
                        )
                    return go

                def masks():
                    # ones/zeros padding of the V2 pair blocks (once per tile)
                    v2 = V2[st * 4 + sub]
                    vv = v2[:].rearrange("p (pr h m) -> p pr h m", pr=4, h=2)
                    nc.gpsimd.memset(vv[:, :, 0, 64:128], 0.0)
                    nc.gpsimd.memset(vv[:, :, 1, 0:64], 0.0)
                    nc.gpsimd.memset(vv[:, :, 0, 64:65], 1.0)
                    nc.gpsimd.memset(vv[:, :, 1, 0:1], 1.0)

                def copy():
                    v2 = V2[st * 4 + sub]
                    vv = v2[:].rearrange("p (pr h m) -> p pr h m", pr=4, h=2)
                    src = holder["ps"][:].rearrange(
                        "p (pr h c) -> p pr h c", pr=4, h=2)
                    nc.vector.tensor_copy(vv[:, :, 0, 0:64], src[:, :, 0, :])
                    nc.vector.tensor_copy(vv[:, :, 1, 64:128], src[:, :, 1, :])

                return [masks] + [mk_mm(kt) for kt in range(KT8)] + [copy]

            for ot in range(NP):
                items.append(qk_group(ot, wq_t, QT, f"pq{st}{ot}"))
                items.append(qk_group(ot, wk_t, KTt, f"pk{st}{ot}"))
            for sub in range(4):
                items.append(v_group(sub))
            return items

        def drain_pair(rec):
            """Drain a finished pair's ctx PSUM banks: ctx halves to SBUF
            (lane-aligned by construction), l rows via ScalarE exp(-ln(l)).
            Invoked after the NEXT pair's first scores are emitted so these
            ops overlap its first exp window."""
            ctxA, ctxB, label = rec["ctxA"], rec["ctxB"], rec["label"]
            cnsrc = ctxraw_pool.tile([128, 512], F32, tag="cr",
                                     name=f"cr{label}")
            lnb = rcp_pool.tile([65, 512], F32, tag="ln", name=f"ln{label}")
            nc.scalar.activation(lnb[64:65, :], ctxA[64:65, :], AF.Ln)
            nc.scalar.activation(lnb[0:1, :], ctxB[0:1, :], AF.Ln)
            nc.vector.tensor_copy(cnsrc[0:64, :], ctxA[0:64, :])
            nc.vector.tensor_copy(cnsrc[64:128, :], ctxB[64:128, :])
            rcp = rcp_pool.tile([65, 512], BF16, tag="rcp",
                                name=f"rcp{label}")
            nc.scalar.activation(rcp[:], lnb[:], AF.Exp, scale=-1.0)
            return (rcp, cnsrc, rec["label"], rec["sink"])

        def norm_part2(rcp, cnsrc, label, sink):
            """Broadcast 1/l across partitions (selector matmul) and scale
            the packed ctx tile straight off the broadcast PSUM."""
            bcp = mm_ps.tile([128, 512], F32, tag="mm", name=f"bcp{label}")
            nc.tensor.matmul(bcp[:], bcsel[:], rcp[:], start=True, stop=True)
            cn = ctxn_pool.tile([128, 512], BF16, tag="cn", name=f"cn{label}")
            nc.vector.tensor_mul(cn[:], cnsrc[:], bcp[:])
            sink.append(cn)

        def attention_block(j, fill, drainq, carried_norm, sink):
            """Causal attention + partial output projection for q-tile j.
            `fill` is a list of closures (next chunk's projection groups)
            sprinkled into the PE stream to cover exp-wait stalls.
            `carried_norm` is the previous block's unemitted normalize; the
            one left over here is returned for the next block, so the PE
            stream never stalls on a normalize chain at a block boundary."""
            fill = [list(g) for g in fill if g]
            n_triples = NP * 4 * (j + 1)
            n_items = sum(len(g) for g in fill)
            per_triple = -(-n_items // n_triples) if n_items else 0

            cur = []  # the single group currently being emitted

            def emit_fill(n):
                # emit up to n single-instruction fill items. Only ONE group
                # is ever partially emitted (it may hold an mm_ps bank; two
                # at once deadlocks against the normalize chain's bcp), but
                # a blocked group is never STARTED — the first ready group
                # is picked instead, so a stuck output-projection head can't
                # starve ready projection groups behind it.
                while n > 0:
                    if not cur:
                        for g in fill:
                            if g[0]() is not False:
                                g.pop(0)
                                n -= 1
                                fill.remove(g)
                                cur.append(g)
                                break
                        else:
                            return  # nothing ready to start
                        if cur and not cur[0]:
                            cur.clear()
                        continue
                    g = cur[0]
                    if not g:
                        cur.clear()
                        continue
                    if g[0]() is False:
                        return  # current group blocked mid-flight
                    g.pop(0)
                    n -= 1
                    if not g:
                        cur.clear()

            def scores(pair, j, i):
                sc = sc_ps.tile([128, 1024], F32, tag="sc",
                                name=f"sc{j}{pair}{i}")
                qa = QT[pair][0:64, j * 512:(j + 1) * 512]
                qb = QT[pair][64:128, j * 512:(j + 1) * 512]
                ka = KTt[pair][0:64, i * 128:(i + 1) * 128]
                kb = KTt[pair][64:128, i * 128:(i + 1) * 128]
                nc.tensor.matmul(
                    sc[:, 0:512], ka, qa,
                    start=True, stop=True, tile_position=(0, 0),
                )
                nc.tensor.matmul(
                    sc[:, 512:1024], kb, qb,
                    start=True, stop=True, tile_position=(64, 0),
                )
                return sc

            ctxn = sink
            ni = 4 * (j + 1)

            def emit_ctx(ctxA, ctxB, et, i):
                first, last = (i == 0), (i == ni - 1)
                va = V2[i][:, pair * 256:pair * 256 + 128]
                vb = V2[i][:, pair * 256 + 128:pair * 256 + 256]
                nc.tensor.matmul(ctxA[:], va, et[:, 0:512],
                                 start=first, stop=last)
                nc.tensor.matmul(ctxB[:], vb, et[:, 512:1024],
                                 start=first, stop=last)

            def emit_exp(sc, i):
                et = exp_pool.tile([128, 1024], BF16, tag="exp",
                                   name=f"et{j}{pair}{i}")
                nc.scalar.activation(et[:], sc[:], AF.Exp)
                if i >= 4 * j:
                    # diagonal block: zero the future positions
                    # keep et[kk, h, qq] iff qq - kk - 128*(i-4j) >= 0
                    p = i - 4 * j
                    nc.gpsimd.affine_select(
                        out=et[:], in_=et[:],
                        pattern=[[0, 2], [1, 512]],
                        compare_op=ALU.is_ge,
                        fill=0.0,
                        base=-128 * p,
                        channel_multiplier=-1,
                    )
                return et

            # i-tiles are processed in groups of two so the scores quadrant
            # phase is entered half as often, and each group's PE stream is
            # [ctx pair x2 | fills | scores x2] — ctx and fills merge into
            # one uninterrupted full-array run.
            pending_norm = carried_norm
            for pair in range(NP):
                ctxA = ctx_ps.tile([128, 512], F32, tag="ctxA",
                                   name=f"ctxA{j}{pair}")
                ctxB = ctx_ps.tile([128, 512], F32, tag="ctxB",
                                   name=f"ctxB{j}{pair}")
                scs = [scores(pair, j, 0), scores(pair, j, 1)]
                # drain the previous pair's ctx banks here, AFTER this pair's
                # first scores are in the PE queue, then emit the normalize
                # for the pair before that
                if drainq:
                    rec = drainq.pop(0)
                    new_norm = drain_pair(rec)
                    if pending_norm is not None:
                        norm_part2(*pending_norm)
                    pending_norm = new_norm
                pending = []
                for ig in range(0, ni, 2):
                    pending.append((emit_exp(scs[0], ig), ig))
                    pending.append((emit_exp(scs[1], ig + 1), ig + 1))
                    while len(pending) > 2:
                        emit_ctx(ctxA, ctxB, *pending.pop(0))
                    emit_fill(2 * per_triple + (per_triple if ig == 0 else 0))
                    if ig + 2 < ni:
                        scs = [scores(pair, j, ig + 2),
                               scores(pair, j, ig + 3)]
                while pending:
                    emit_ctx(ctxA, ctxB, *pending.pop(0))
                    emit_fill(per_triple)
                drainq.append(dict(ctxA=ctxA, ctxB=ctxB,
                                   label=f"{j}{pair}", sink=ctxn))

            # drain any remaining fill that's ready; groups still blocked on
            # the not-yet-flushed drain queue are returned to the caller
            if cur and cur[0]:
                fill.insert(0, cur[0])
            progress = True
            while fill and progress:
                progress = False
                g = fill[0]
                while g and g[0]() is not False:
                    g.pop(0)
                    progress = True
                if not g:
                    fill.pop(0)
                    progress = True
            return pending_norm, fill

        def outproj_items(j, ctxn):
            """Output projection for q-tile j as fine-grain fill items."""
            items = []

            def group(s4, oh, holder):
                def mk_mm(pair):
                    def go():
                        if len(ctxn) <= pair:
                            return False  # cn not normalized yet
                        if "ps" not in holder:
                            holder["ps"] = mm_ps.tile(
                                [128, 512], F32, tag="mm", name=f"yp{j}{s4}{oh}")
                        nc.tensor.matmul(
                            holder["ps"][:],
                            ctxn[pair][:, s4 * 128:(s4 + 1) * 128],
                            wo_t[pair][:, oh * 512:(oh + 1) * 512],
                            start=(pair == 0),
                            stop=(pair == NP - 1),
                        )
                    return go

                def copy():
                    nc.vector.tensor_copy(
                        holder["yb"][:, oh * 512:(oh + 1) * 512], holder["ps"][:])
                    del holder["ps"]

                return [mk_mm(p) for p in range(NP)] + [copy]

            for s4 in range(4):
                srow = j * 4 + s4
                holder = {}

                def alloc_yb(holder=holder, s4=s4):
                    holder["yb"] = ybuf_pool.tile(
                        [128, D], F32, tag="yb", name=f"yb{j}{s4}")

                g = [alloc_yb]
                for oh in range(2):
                    g.extend(group(s4, oh, holder))

                def dma_out(holder=holder, srow=srow):
                    nc.sync.dma_start(
                        yout[srow * 128:(srow + 1) * 128, :], holder["yb"][:])

                g.append(dma_out)
                items.append(g)
            return items

        # chunk 0 projections run alone; attention block j then carries
        # chunk j+1's projections and block j-1's output projection as PE
        # filler for its exp-wait stalls.
        xts0 = emit_xt_dmas(0)
        for kt in range(KT8):
            for pool, lst, srcp, nm in (
                (wk_pool, wk_t, wkT, "wk"),
                (wv_pool, wv_t, wvT, "wv"),
            ):
                t = pool.tile([128, HG], BF16, tag=f"w{kt}", name=f"{nm}{kt}")
                nc.sync.dma_start(t[:], srcp[kt * 128:(kt + 1) * 128, :])
                lst.append(t)
        for g0 in proj_items(0, xts0):
            for item in g0:
                item()
        wo_t.clear()
        for c in range(NP):
            t = wo_pool.tile([128, D], BF16, tag=f"wo{c}")
            nc.sync.dma_start(t[:], woT[c * 128:(c + 1) * 128, :])
            wo_t.append(t)
        prev_out = []
        drainq = []
        carried = None
        sinks = [[] for _ in range(4)]
        for j in range(4):
            fill = list(prev_out)
            if j + 1 < 4:
                xts = emit_xt_dmas(j + 1)
                fill = proj_items(j + 1, xts) + fill
            else:
                # last block: interleave its own output projection as fill
                # (items block until each pair's cn lands, then emit)
                fill = fill + outproj_items(j, sinks[j])
            carried, leftover = attention_block(
                j, fill, drainq, carried, sinks[j])
            if j + 1 < 4:
                assert not leftover
                prev_out = outproj_items(j, sinks[j])
            else:
                prev_out = leftover
        # flush: the last pair's drain and the final two normalizes
        while drainq:
            new_norm = drain_pair(drainq.pop(0))
            if carried is not None:
                norm_part2(*carried)
            carried = new_norm
        if carried is not None:
            norm_part2(*carried)
        for g in prev_out:
            for item in g:
                assert item() is not False


    _enforce_wait_limits(nc.m)
    return nc


_NC = None


def _get_nc():
    global _NC
    if _NC is None:
        _NC = build_nc()
    return _NC


def run(x, Wq, Wk, Wv, Wo, trace=False, trace_kwargs=None):
    """Returns (y, BassKernelResults)."""
    x = np.asarray(x, np.float32)
    scale = 1.0 / np.sqrt(DK)
    in_maps = []
    for core in range(N_CORES):
        b, g = core // 2, core % 2
        cols = slice(g * HG, (g + 1) * HG)
        bf = ml_dtypes.bfloat16
        in_maps.append({
            "xT": np.ascontiguousarray(x[b].T).astype(bf),
            "wqT": np.ascontiguousarray(
                np.asarray(Wq, np.float32).T[:, cols] * scale).astype(bf),
            "wkT": np.ascontiguousarray(
                np.asarray(Wk, np.float32).T[:, cols]).astype(bf),
            "wvT": np.ascontiguousarray(
                np.asarray(Wv, np.float32).T[:, cols]).astype(bf),
            "woT": np.ascontiguousarray(
                np.asarray(Wo, np.float32).T[cols, :]).astype(bf),
        })
    kw = dict(trace_kwargs or {})
    res = run_bass_kernel_spmd(
        _get_nc(), in_maps, list(range(N_CORES)), trace=trace, **kw
    )
    y = np.empty((B, S, D), np.float32)
    for b in range(B):
        y[b] = res.results[2 * b]["y"] + res.results[2 * b + 1]["y"]
    return y, res


def kernel(x, Wq, Wk, Wv, Wo):
    y, _ = run(x, Wq, Wk, Wv, Wo)
    return y

